# revision 36
# baseline (speedup 1.0000x reference)
# CapsuleNetwork Trainium2 kernel (8-core data parallel, 4 images/core).
#
# Per core:
#   conv1 3->256 k9 s1 (im2col K=243, bf16 matmuls) + relu
#   conv2 256->256 k9 s2 (81-tap PSUM accumulation, bf16, weights resident)
#   capsule squash (pixel-major), dynamic routing (3 iters) without
#   materializing u_hat:
#     F'[(r,i),(r',c)] = sum_p capsW[p,(r,i)] * exp(b)[p,(r',c)]; G = diag blocks
#     s[c,o]  = sum_{r,i} G[r,c,i] * route_w[r,c,i,o]   (per-class matmuls)
#     b     += caps @ T_block,  T_block[(r,i),(r,c)] = sum_o w[r,c,i,o] v[c,o]
import functools
from contextlib import ExitStack

import numpy as np
import ml_dtypes

import concourse.bass as bass
import concourse.tile as tile
from concourse import bacc
from concourse import mybir
from concourse.bass_utils import run_bass_kernel_spmd

BF = mybir.dt.bfloat16
E4 = mybir.dt.float8e4
F32 = mybir.dt.float32
AF = mybir.ActivationFunctionType
AX = mybir.AxisListType

NCORES = 8
B = 4              # images per core
K1 = 243           # 3*9*9 im2col contraction
NPIX1 = 3136       # 56*56 conv1 output pixels
N1CH = 448         # conv1 moving chunk (3136 = 7*448)
PIX = 576          # 24*24 conv2 output pixels
PIXP = 640         # padded to 5*128 so XBAR DMA transposes are legal
PIX_CHUNKS = [(0, 128), (128, 128), (256, 128), (384, 128), (512, 128)]
R, D, C, O = 32, 8, 10, 16


def _build_nc():
    nc = bacc.Bacc("TRN2", target_bir_lowering=False, debug=False)
    # register the squash-eps constant for activation bias use
    eps_t = nc.alloc_sbuf_tensor("const-eps", [128, 1], F32)
    nc.gpsimd.memset(eps_t.ap(), 1e-8)
    nc.const_aps.aps[(F32, 1e-8)] = eps_t.ap()
    nc.all_engine_barrier()
    # x im2col, fp8, 2 k-tiles packed per partition: [b, p, t, pix] = row t*128+p
    x_d = nc.declare_dram_parameter("x", [B, 128, 2, NPIX1], E4, isOutput=False)
    w1_d = nc.declare_dram_parameter("w1", [256, 256], BF, isOutput=False)
    b1_d = nc.declare_dram_parameter("b1", [256, 1], F32, isOutput=False)
    w2_d = nc.declare_dram_parameter("w2", [2, 2, 128, 81 * 128], BF, isOutput=False)
    b2_d = nc.declare_dram_parameter("b2", [256, 1], F32, isOutput=False)
    ws_d = nc.declare_dram_parameter("ws", [256, C * O], BF, isOutput=False)
    wcob_d = nc.declare_dram_parameter("wcob", [O, C, 256], BF, isOutput=False)
    maskg_d = nc.declare_dram_parameter("maskg", [2, 128, R * C], F32, isOutput=False)
    idf_d = nc.declare_dram_parameter("idf", [128, 128], F32, isOutput=False)
    idb_d = nc.declare_dram_parameter("idb", [128, 128], BF, isOutput=False)
    vout_d = nc.declare_dram_parameter("v_out", [B * C, O], F32, isOutput=True)

    with tile.TileContext(nc) as tc, ExitStack() as ctx:
        consts = ctx.enter_context(tc.tile_pool(name="consts", bufs=1))
        w1a = consts.tile([128, 256], BF, tag="w1a", name="w1a")
        w1b = consts.tile([128, 256], BF, tag="w1b", name="w1b")
        nc.sync.dma_start(w1a, w1_d[0:128, :])
        nc.sync.dma_start(w1b, w1_d[128:256, :])
        b1t = [consts.tile([128, 1], F32, tag=f"b1_{m}", name=f"b1_{m}") for m in range(2)]
        b2t = [consts.tile([128, 1], F32, tag=f"b2_{m}", name=f"b2_{m}") for m in range(2)]
        for m in range(2):
            nc.sync.dma_start(b1t[m], b1_d[m * 128:(m + 1) * 128, :])
            nc.sync.dma_start(b2t[m], b2_d[m * 128:(m + 1) * 128, :])
        ws_t = [consts.tile([128, C * O], BF, tag=f"ws{m}", name=f"ws{m}") for m in range(2)]
        for m in range(2):
            nc.scalar.dma_start(ws_t[m], ws_d[m * 128:(m + 1) * 128, :])
        wcob = consts.tile([O, C, 256], BF, tag="wcob", name="wcob")
        nc.scalar.dma_start(wcob, wcob_d[:, :, :])
        idf = consts.tile([128, 128], F32, tag="idf", name="idf")
        idb = consts.tile([128, 128], BF, tag="idb", name="idb")
        nc.sync.dma_start(idf, idf_d[:, :])
        nc.scalar.dma_start(idb, idb_d[:, :])
        # block-diag masks: maskg[m][j, r*C+c] = (r == m*16 + j//8)
        maskg = [consts.tile([128, R * C], F32, tag=f"mg{m}", name=f"mg{m}")
                 for m in range(2)]
        for m in range(2):
            nc.scalar.dma_start(maskg[m], maskg_d[m])
        ones16 = consts.tile([16, 1], F32, tag="ones16", name="ones16")
        ones1 = consts.tile([1, 16], F32, tag="ones1", name="ones1")
        nc.vector.memset(ones16, 1.0)
        nc.vector.memset(ones1, 1.0)

        # ---- persistent caps tensors (written during conv phase) ----
        persist = ctx.enter_context(tc.tile_pool(name="persist", bufs=1))
        caps_bf = [persist.tile([128, B, 256], BF, tag=f"cbf{k}", name=f"cbf{k}")
                   for k in range(5)]                              # pixel-major squashed
        capsT = [[persist.tile([128, PIXP], BF, tag=f"cT{b}_{g}", name=f"cT{b}_{g}")
                  for g in range(2)] for b in range(B)]            # channel-major squashed
        capsum = [persist.tile([128, B], F32, tag=f"cs{g}", name=f"cs{g}")
                  for g in range(2)]
        rpool = ctx.enter_context(tc.tile_pool(name="rpool", bufs=1))
        dpool = ctx.enter_context(tc.tile_pool(name="dtmp", bufs=4))
        sfx = ctx.enter_context(tc.tile_pool(name="sfx", bufs=2))
        blog = [rpool.tile([128, B, R, C], F32, tag=f"bl{k}", name=f"bl{k}")
                for k in range(5)]
        capsum_bf = [rpool.tile([128, B], BF, tag=f"csb{g}", name=f"csb{g}")
                     for g in range(2)]

        # ================= conv1 + conv2 + squash =================
        with tc.tile_pool(name="w2pool", bufs=1) as w2pool, \
             tc.tile_pool(name="h1pool", bufs=1) as h1pool:
            w2t = [[w2pool.tile([128, 81, 128], BF, tag=f"w2_{ig}_{og}", name=f"w2_{ig}_{og}")
                    for og in range(2)] for ig in range(2)]
            # h1 phase-split along x so conv2 rhs reads are stride-1:
            # [128, b, y(56), phase(2), x'(28)]; fp8 (rel-err impact ~1e-3)
            h1 = [h1pool.tile([128, B, 56, 2, 28], E4, tag=f"h1_{g}", name=f"h1_{g}")
                  for g in range(2)]

            # ---- conv1 ----
            with tc.tile_pool(name="imcol", bufs=1) as impool, \
                 tc.tile_pool(name="c1psum", bufs=6, space="PSUM") as c1psum:
                # x im2col first on the 16-engine SWDGE queue (conv1 critical
                # path), then w2 in 27-tap chunks ordered by consumption.
                im8 = [impool.tile([128, 2, NPIX1], E4, tag=f"im{b}", name=f"im{b}")
                       for b in range(B)]

                def w2_dma(og):
                    for ig in range(2):
                        for cc in range(3):
                            nc.gpsimd.dma_start(
                                w2t[ig][og][:, 27 * cc:27 * (cc + 1), :]
                                .rearrange("p t m -> p (t m)"),
                                w2_d[ig, og][:, 27 * cc * 128:(27 * cc + 27) * 128])

                # SWDGE FIFO order = consumption order: all images first
                # (conv1 must never stall — it heads the in-order PE queue),
                # then w2 og0 (conv2 start gate), then og1
                for b in range(B):
                    nc.gpsimd.dma_start(im8[b], x_d[b])
                w2_dma(0)
                w2_dma(1)
                # p-state warmup: junk matmuls keep the PE streaming while
                # the first image's im2col DMA lands
                with tc.tile_pool(name="warm", bufs=2, space="PSUM") as warm:
                    for _ in range(12):
                        wps = warm.tile([128, 256], F32, tag="warm", name="warm")
                        nc.tensor.matmul(wps, w1a[:, 0:128], w1a,
                                         start=True, stop=True)
                for b in range(B):
                    for m in range(2):
                        for n in range(7):  # 448 pixels = 8 rows of 56
                            ps = c1psum.tile([128, 8, 56], F32, tag="c1ps", name="c1ps")
                            nc.tensor.matmul(ps, w1a[:, m * 128:(m + 1) * 128],
                                             im8[b][:, 0, n * N1CH:(n + 1) * N1CH],
                                             start=True, stop=False)
                            nc.tensor.matmul(ps, w1b[:, m * 128:(m + 1) * 128],
                                             im8[b][:, 1, n * N1CH:(n + 1) * N1CH],
                                             start=False, stop=True)
                            for px in range(2):
                                eng = nc.scalar if px == 0 else nc.vector
                                if px == 0:
                                    nc.scalar.activation(
                                        h1[m][:, b, 8 * n:8 * n + 8, px, :],
                                        ps[:, :, px::2],
                                        AF.Relu, bias=b1t[m], scale=1.0)
                                else:
                                    # relu(x + bias) on DVE to split drain load
                                    nc.vector.tensor_scalar(
                                        h1[m][:, b, 8 * n:8 * n + 8, px, :],
                                        ps[:, :, px::2], b1t[m], 0.0,
                                        op0=mybir.AluOpType.add,
                                        op1=mybir.AluOpType.max)

            # ---- conv2 + squash + routing, pipelined per image:
            # routing of image b (mostly DVE/Scalar + tiny matmuls) overlaps
            # the conv2 tap-accumulation of image b+1 on the PE. ----
            with tc.tile_pool(name="craw", bufs=2) as crawpool, \
                 tc.tile_pool(name="c2psum", bufs=3, space="PSUM") as c2psum, \
                 tc.tile_pool(name="tposum", bufs=1, space="PSUM") as tposum, \
                 tc.tile_pool(name="dps", bufs=2, space="PSUM") as dps, \
                 tc.tile_pool(name="rpsum", bufs=1, space="PSUM") as rpsum, \
                 tc.tile_pool(name="pmraw", bufs=2) as pmpool, \
                 tc.tile_pool(name="sqtmp", bufs=4) as sqpool:
                # all small routing psum tensors share one 2KB bank (psum
                # pool tiles are bank-granular)
                rps = rpsum.tile([128, 512], F32, tag="rps", name="rps")
                F4v = rps[:, 0:320]
                t4v = rps[:, 320:330]
                s4Tv = rps[0:16, 330:340]
                nsqrv = rps[0:1, 340:350]
                sgTv = rps[0:16, 350:360]
                s4v = rps[0:C, 360:376]

                def v_squash_img(s4T, b, last):
                    """s4T: psum [16, C] -> v4T bf16 [16, C] (or writes v_out)."""
                    s4T_sb = dpool.tile([16, C], F32, tag="s4Tsb", name="s4Tsb")
                    nc.vector.tensor_copy(s4T_sb, s4T)
                    if last:
                        s4p = s4v
                        nc.tensor.transpose(s4p, s4T_sb, idf[:16, :16])
                        s4 = dpool.tile([C, 16], F32, tag="s4sb", name="s4sb")
                        nc.vector.tensor_copy(s4, s4p)
                        sq = dpool.tile([C, 16], F32, tag="vsq", name="vsq")
                        nc.vector.tensor_mul(sq, s4, s4)
                        nsq = dpool.tile([C, 1], F32, tag="vnsq", name="vnsq")
                        nc.vector.reduce_sum(nsq, sq, axis=AX.X)
                        a = dpool.tile([C, 1], F32, tag="va", name="va")
                        nc.scalar.activation(a, nsq, AF.Sqrt, bias=1e-8)
                        nc.vector.scalar_tensor_tensor(
                            a, nsq, 1.0, a,
                            op0=mybir.AluOpType.add, op1=mybir.AluOpType.mult)
                        nc.vector.reciprocal(a, a)
                        nc.vector.tensor_mul(a, nsq, a)
                        vout = dpool.tile([C, 16], F32, tag="vout", name="vout")
                        nc.vector.tensor_mul(vout, s4, a.broadcast_to([C, 16]))
                        nc.sync.dma_start(vout_d[b * C:(b + 1) * C, :], vout)
                        return None
                    # row-major squash: partition-reduce |s|^2 via ones-matmul,
                    # broadcast the scale back via a K=1 outer-product matmul.
                    # square from the SBUF copy, NOT the psum tensor: the nsqr
                    # matmul's start=True marks the whole shared psum bank
                    # pending-zero, so every read of s4T must be ordered
                    # before it via the s4T_sb dependency chain.
                    sqT = dpool.tile([16, C], F32, tag="vsqT", name="vsqT")
                    nc.vector.tensor_mul(sqT, s4T_sb, s4T_sb)
                    nsqr = nsqrv
                    nc.tensor.matmul(nsqr, ones16, sqT, start=True, stop=True)
                    a = dpool.tile([1, C], F32, tag="var", name="var")
                    nc.scalar.activation(a, nsqr, AF.Sqrt, bias=1e-8)
                    nc.vector.scalar_tensor_tensor(
                        a, nsqr, 1.0, a,
                        op0=mybir.AluOpType.add, op1=mybir.AluOpType.mult)
                    nc.vector.reciprocal(a, a)
                    sgr = dpool.tile([1, C], F32, tag="sgr", name="sgr")
                    nc.vector.tensor_mul(sgr, nsqr, a)
                    sgT = sgTv
                    nc.tensor.matmul(sgT, ones1, sgr, start=True, stop=True)
                    v4T = dpool.tile([16, C], BF, tag="v4T", name="v4T")
                    nc.vector.tensor_mul(v4T, s4T_sb, sgT)
                    return v4T

                def b_update_img(v4T, b, it):
                    """b_log[:, b] += caps . T_block (T = route_w . v)."""
                    # dummy exp: prefetch the EXP act-table (1.3us load, evicted
                    # by SQRT) under this phase's matmuls, off the next
                    # softmax's critical path
                    warmx = dpool.tile([1, 1], F32, tag="warmx", name="warmx")
                    nc.scalar.activation(warmx, ones1[:, 0:1], AF.Exp)
                    T4b = []
                    for m in range(2):
                        t4 = t4v
                        for c in range(C):
                            nc.tensor.matmul(t4[:, c:c + 1],
                                             wcob[:, c, m * 128:(m + 1) * 128],
                                             v4T[:, c:c + 1],
                                             start=True, stop=True)
                        T4m = dpool.tile([128, R, C], BF, tag=f"T4_{m}",
                                         name=f"T4_{m}")
                        nc.vector.tensor_mul(
                            T4m,
                            t4.unsqueeze(1).broadcast_to([128, R, C]),
                            maskg[m].rearrange("p (r c) -> p r c", c=C))
                        T4b.append(T4m.rearrange("p r c -> p (r c)"))
                    for k, (p0, ln) in enumerate(PIX_CHUNKS):
                        dl = dps.tile([128, R, C], F32, tag="dl", name="dl")
                        for kc in range(2):
                            nc.tensor.matmul(dl[:ln],
                                             capsT[b][kc][:, p0:p0 + ln],
                                             T4b[kc],
                                             start=(kc == 0), stop=(kc == 1))
                        if it == 0:
                            nc.vector.tensor_copy(blog[k][:ln, b], dl[:ln])
                        else:
                            nc.vector.tensor_add(blog[k][:ln, b],
                                                 blog[k][:ln, b], dl[:ln])

                def softmax_G_img(b):
                    """softmax over c folded into caps; G = diag of cw.T @ e."""
                    e, cw = [], []
                    for k, (p0, ln) in enumerate(PIX_CHUNKS):
                        et = sfx.tile([128, R, C], BF, tag=f"e{k}", name=f"e{k}")
                        nc.scalar.activation(et[:ln], blog[k][:ln, b], AF.Exp)
                        den = dpool.tile([128, R], F32, tag="den", name="den")
                        nc.vector.reduce_sum(den[:ln], et[:ln], axis=AX.X)
                        nc.vector.reciprocal(den[:ln], den[:ln])
                        cwt = sfx.tile([128, R, D], BF, tag=f"cw{k}", name=f"cw{k}")
                        cbf4 = caps_bf[k][:, b].rearrange("p (r i) -> p r i", i=D)
                        nc.vector.tensor_mul(
                            cwt[:ln], cbf4[:ln],
                            den[:ln].unsqueeze(2).broadcast_to([ln, R, D]))
                        e.append(et)
                        cw.append(cwt)
                    Gp = []
                    for m in range(2):
                        F4 = F4v
                        for k, (p0, ln) in enumerate(PIX_CHUNKS):
                            cwf = cw[k].rearrange("p r i -> p (r i)")
                            ef = e[k].rearrange("p r c -> p (r c)")
                            nc.tensor.matmul(F4,
                                             cwf[:ln, m * 128:(m + 1) * 128],
                                             ef[:ln],
                                             start=(k == 0), stop=(k == 4))
                        fm = dpool.tile([128, R * C], BF, tag="fm", name="fm")
                        nc.vector.tensor_mul(fm, F4, maskg[m])
                        gf = dpool.tile([128, C], F32, tag="gf", name="gf")
                        nc.vector.reduce_sum(
                            gf, fm.rearrange("p (r c) -> p c r", c=C), axis=AX.X)
                        Gpm = dpool.tile([128, C], BF, tag=f"G{m}", name=f"G{m}")
                        nc.vector.tensor_copy(Gpm, gf)
                        Gp.append(Gpm)
                    return Gp

                def s_matmuls_img(rhs_pair):
                    # dummy sqrt: prefetch the SQRT act-table (evicted by EXP)
                    # under the s matmuls, off v_squash's critical path
                    warms = dpool.tile([1, 1], F32, tag="warms", name="warms")
                    nc.scalar.activation(warms, ones1[:, 0:1], AF.Sqrt, bias=1e-8)
                    s4T = s4Tv
                    for c in range(C):
                        for m in range(2):
                            rhs = rhs_pair[m]
                            if rhs.shape[-1] == C:
                                rhs = rhs[:, c:c + 1]
                            nc.tensor.matmul(s4T[:, c:c + 1],
                                             ws_t[m][:, c * 16:(c + 1) * 16],
                                             rhs, start=(m == 0), stop=(m == 1))
                    return s4T

                for b in range(B):
                    capsT_raw = [crawpool.tile([128, PIXP], BF, tag=f"cr{g}",
                                               name=f"cr{g}") for g in range(2)]
                    for og in range(2):
                        # zero pad columns so padded-chunk squash yields zeros
                        nc.vector.memset(capsT_raw[og][:, PIX:PIXP], 0.0)
                        pss = [c2psum.tile([128, 288], F32, tag="c2ps", name="c2ps")
                               for _ in range(2)]
                        for ig in range(2):
                            for t81 in range(81):
                                kh, kw = t81 // 9, t81 % 9
                                lhsT = w2t[ig][og][:, t81, :]
                                for y in range(2):
                                    rhs = h1[ig][:, b,
                                                 kh + 24 * y:kh + 24 * y + 24:2,
                                                 kw % 2, kw // 2:kw // 2 + 24]
                                    nc.tensor.matmul(
                                        pss[y], lhsT, rhs,
                                        start=(ig == 0 and t81 == 0),
                                        stop=(ig == 1 and t81 == 80))
                        for y in range(2):
                            nc.vector.tensor_scalar(
                                capsT_raw[og][:, y * 288:(y + 1) * 288], pss[y],
                                b2t[og], 0.0,
                                op0=mybir.AluOpType.add,
                                op1=mybir.AluOpType.bypass)
                    # pixel-major squash
                    for k, (p0, ln) in enumerate(PIX_CHUNKS):
                        pm = pmpool.tile([128, 256], BF, tag="pm", name="pm")
                        for og in range(2):
                            tp = tposum.tile([128, 128], BF, tag="tp", name="tp")
                            nc.tensor.transpose(tp, capsT_raw[og][:, p0:p0 + ln],
                                                idb)
                            nc.vector.tensor_copy(
                                pm[:, og * 128:(og + 1) * 128], tp)
                        pm3 = pm.rearrange("p (r i) -> p r i", i=D)
                        sq = sqpool.tile([128, R, D], F32, tag="sq", name="sq")
                        nc.vector.tensor_mul(sq[:ln], pm3[:ln], pm3[:ln])
                        nsq = sqpool.tile([128, R], F32, tag="nsq", name="nsq")
                        nc.vector.reduce_sum(nsq[:ln], sq[:ln], axis=AX.X)
                        a = sqpool.tile([128, R], F32, tag="sqa", name="sqa")
                        nc.scalar.activation(a[:ln], nsq[:ln], AF.Sqrt, bias=1e-8)
                        nc.vector.scalar_tensor_tensor(
                            a[:ln], nsq[:ln], 1.0, a[:ln],
                            op0=mybir.AluOpType.add, op1=mybir.AluOpType.mult)
                        nc.vector.reciprocal(a[:ln], a[:ln])
                        nc.vector.tensor_mul(a[:ln], nsq[:ln], a[:ln])
                        cbf3 = caps_bf[k][:, b].rearrange("p (r i) -> p r i", i=D)
                        nc.vector.tensor_mul(
                            cbf3[:ln], pm3[:ln],
                            a[:ln].unsqueeze(2).broadcast_to([ln, R, D]))
                        for og in range(2):
                            tb = tposum.tile([128, 128], BF, tag="tb", name="tb")
                            nc.tensor.transpose(
                                tb, caps_bf[k][:ln, b, og * 128:(og + 1) * 128],
                                idb)
                            nc.vector.tensor_copy(capsT[b][og][:, p0:p0 + ln],
                                                  tb)
                    for g in range(2):  # iter-0 capsule sums
                        nc.vector.reduce_sum(capsum[g][:, b:b + 1], capsT[b][g],
                                             axis=AX.X)

                    # ---- routing for image b (overlaps conv2 of b+1) ----
                    for g in range(2):
                        nc.vector.tensor_scalar_mul(capsum_bf[g][:, b:b + 1],
                                                    capsum[g][:, b:b + 1],
                                                    1.0 / C)
                    s4T = s_matmuls_img([capsum_bf[0][:, b:b + 1],
                                         capsum_bf[1][:, b:b + 1]])
                    v4T = v_squash_img(s4T, b, last=False)
                    b_update_img(v4T, b, it=0)
                    for it in (1, 2):
                        Gp = softmax_G_img(b)
                        s4T = s_matmuls_img(Gp)
                        v4T = v_squash_img(s4T, b, last=(it == 2))
                        if it == 1:
                            b_update_img(v4T, b, it=1)

    nc.compile()
    return nc


@functools.lru_cache(maxsize=1)
def _get_nc():
    return _build_nc()


def _prep_consts(conv1_w, conv1_b, conv2_w, conv2_b, route_w):
    bf = ml_dtypes.bfloat16
    f32 = np.float32
    w1 = np.zeros((256, 256), f32)
    w1[:K1] = conv1_w.astype(f32).transpose(1, 2, 3, 0).reshape(K1, 256)
    w2 = (conv2_w.astype(f32)
          .reshape(2, 128, 2, 128, 81)       # [og, mo, ig, ki, tap]
          .transpose(2, 0, 3, 4, 1))         # [ig, og, ki, tap, mo] (contiguous DMA)
    ws = route_w.astype(f32).transpose(0, 2, 1, 3).reshape(256, C * O)
    wcob = route_w.astype(f32).transpose(3, 1, 0, 2).reshape(O, C, 256)
    maskg = np.zeros((2, 128, R * C), f32)
    for m in range(2):
        for j in range(128):
            r = m * 16 + j // D
            maskg[m, j, r * C:(r + 1) * C] = 1.0
    return {
        "w1": np.ascontiguousarray(w1).astype(bf),
        "b1": np.ascontiguousarray(conv1_b.astype(f32).reshape(256, 1)),
        "w2": np.ascontiguousarray(w2).reshape(2, 2, 128, 81 * 128).astype(bf),
        "b2": np.ascontiguousarray(conv2_b.astype(f32).reshape(256, 1)),
        "ws": np.ascontiguousarray(ws).astype(bf),
        "wcob": np.ascontiguousarray(wcob).astype(bf),
        "idf": np.eye(128, dtype=f32),
        "idb": np.eye(128, dtype=f32).astype(bf),
        "maskg": maskg,
    }


def _ensure_ntff_hook():
    """The agent image's antenv lacks axon_hooks; shim it so trace=True works."""
    import sys
    import types
    try:
        from antenv import axon_hooks  # noqa: F401
        return
    except ImportError:
        pass
    mod = types.ModuleType("antenv.axon_hooks")
    _h = [None]
    mod.get_axon_ntff_profile_hook = lambda: _h[0]
    mod.set_axon_ntff_profile_hook = lambda h: _h.__setitem__(0, h)
    sys.modules["antenv.axon_hooks"] = mod
    try:
        from trn_agent_boot.trn_boot import _ntff_profile_via_ctypes
        mod.set_axon_ntff_profile_hook(
            _ntff_profile_via_ctypes("/opt/axon/libaxon_pjrt.so"))
    except Exception as e:  # degrade: trace skipped, run still works
        print(f"ntff hook shim failed: {e}")


def run(x, conv1_w, conv1_b, conv2_w, conv2_b, route_w, trace=False, cores=NCORES):
    if trace:
        _ensure_ntff_hook()
    x = np.asarray(x, np.float32)
    nb = x.shape[0]
    consts = _prep_consts(np.asarray(conv1_w), np.asarray(conv1_b),
                          np.asarray(conv2_w), np.asarray(conv2_b),
                          np.asarray(route_w))
    win = np.lib.stride_tricks.sliding_window_view(x, (9, 9), axis=(2, 3))
    xb = (win.transpose(0, 1, 4, 5, 2, 3)          # [b, c, kh, kw, y, x]
          .reshape(nb, K1, NPIX1))
    # fp8, padded to 256 rows, 2 k-tiles packed per partition:
    # [b, p, t, pix] = row t*128+p
    xp = np.zeros((nb, 256, NPIX1), np.float32)
    xp[:, :K1] = xb
    xb = (xp.reshape(nb, 2, 128, NPIX1).transpose(0, 2, 1, 3)
          .astype(ml_dtypes.float8_e4m3))
    assert nb == B * cores
    in_maps = []
    for cid in range(cores):
        m = dict(consts)
        m["x"] = np.ascontiguousarray(xb[cid * B:(cid + 1) * B])
        in_maps.append(m)
    res = run_bass_kernel_spmd(_get_nc(), in_maps, list(range(cores)), trace=trace)
    out = np.concatenate([r["v_out"].reshape(B, C, O) for r in res.results], axis=0)
    return out.astype(np.float32), res


def kernel(x, conv1_w, conv1_b, conv2_w, conv2_b, route_w):
    out, _ = run(x, conv1_w, conv1_b, conv2_w, conv2_b, route_w, trace=False)
    return out

